# revision 1
# baseline (speedup 1.0000x reference)
# Trainium2 Bass kernel for nn_DecoderBlock (masked self-attn + cross-attn +
# LFFN decoder block with "linear" softmax attention over the head dim).
#
# Sharding: data-parallel over batch — 16 batch elems / 8 cores = 2 per core.
# All weights replicated per core (bf16); activations stream per batch elem.
#
# Math per core/batch elem (validated against the jax reference in numpy):
#   per head: Q/K/V = x @ W[h]        ([s, dq] layout, s on partitions)
#   expQ/expK = exp((Q|K)/DQ**0.25)   (mask added to Q rows < 127 first)
#   V' = V * (1/rowsum(expK))         (folds K-softmax denominator)
#   A  = expK^T @ V'                  ([dq, dq])
#   softQ = expQ * (1/rowsum(expQ));  softQT = transpose(softQ)   [dq, s]
#   BmT = A^T @ softQT                ([dq, s])
#   out rows [128h:128h+128] = sum_j BmT[:, j::8].T @ Wo.T[128j:128j+128, :]
#     (replicates the module's raw [b,h,s,d] -> [b, s, h*d] view)
#   residual + layernorm in natural [s, D] layout; LFFN via transposed chain.
import numpy as np
import ml_dtypes

import concourse.bacc as bacc
import concourse.mybir as mybir
import concourse.tile as tile
from concourse.bass_utils import run_bass_kernel_spmd

H, D, DQ, BNK, HID = 8, 1024, 128, 512, 1024
B, S_T, S_M = 16, 1024, 2048
SCALE = DQ ** 0.25
EPS = 1e-5
NEG = -200.0
N_CORES = 8
BPC = B // N_CORES  # batch elems per core

f32 = mybir.dt.float32
bf16 = mybir.dt.bfloat16
AF = mybir.ActivationFunctionType
ALU = mybir.AluOpType
bf = ml_dtypes.bfloat16


def _build(affine: bool):
    nc = bacc.Bacc("TRN2", target_bir_lowering=False, debug=False,
                   enable_asserts=True, num_devices=N_CORES)

    dt_in = {}
    def din(name, shape, dt=bf16):
        dt_in[name] = nc.dram_tensor(name, list(shape), dt, kind="ExternalInput").ap()
        return dt_in[name]

    y0 = din("y0", [BPC, S_T, D], f32)
    y0T = din("y0T", [BPC, 8, 128, S_T])           # [b][kchunk][128, S_T] bf16
    memT = din("memT", [BPC, 8, 16, 128, 128])     # [b][kchunk][smtile][128,128]
    wqkv1 = din("wqkv1", [3, 2, 8, 128, 512])      # [qkv][hg][kchunk][128, 512]
    wqkv2 = din("wqkv2", [3, 2, 8, 128, 512])
    wo1t = din("wo1t", [8, 128, D])                # [j][128, D]
    wo2t = din("wo2t", [8, 128, D])
    e1t = din("e1t", [8, 4, 128, 128])             # [kchunk][bn_tile][128,128]
    d1t = din("d1t", [4, 8, 128, 128])             # [bn_chunk][hid_tile]
    e2t = din("e2t", [8, 4, 128, 128])             # [hid_chunk][bn_tile]
    d2t = din("d2t", [4, 128, D])                  # [bn_chunk][128, D]
    mask4 = din("mask4", [128, 512], f32)
    if affine:
        grep = din("grep", [6, 128, D], f32)       # g1,b1,g2,b2,g3,b3 replicated

    out = nc.dram_tensor("out", [BPC, S_T, D], f32, kind="ExternalOutput").ap()

    with tile.TileContext(nc) as tc:
        with tc.tile_pool(name="dram", bufs=1, space="DRAM") as dpool:
            y1d = dpool.tile([BPC, S_T, D], f32)
            y2d = dpool.tile([BPC, S_T, D], f32)

            with tc.tile_pool(name="consts", bufs=1) as cpool:
                maskt = cpool.tile([128, 512], f32, tag="maskt")
                nc.sync.dma_start(maskt[:], mask4[:])
                eps_t = cpool.tile([128, 1], f32, tag="eps_t")
                nc.vector.memset(eps_t[:], EPS)
                gb = None
                if affine:
                    gb = [cpool.tile([128, D], f32, tag=f"gb{i}", name=f"gb{i}") for i in range(6)]
                    for i in range(6):
                        nc.sync.dma_start(gb[i][:], grep[i])

                _phase_attn(nc, tc, b_iter=range(BPC), masked=True,
                            xq_nat=y0, xqT_dram=y0T, kvT_dram=None,
                            wqkv=wqkv1, wot=wo1t, n_kv=8, maskt=maskt,
                            y_next_d=y1d, gb=gb, gbi=0, eps_t=eps_t)
                _phase_attn(nc, tc, b_iter=range(BPC), masked=False,
                            xq_nat=y1d, xqT_dram=None, kvT_dram=memT,
                            wqkv=wqkv2, wot=wo2t, n_kv=16, maskt=None,
                            y_next_d=y2d, gb=gb, gbi=2, eps_t=eps_t)
                _phase_lffn(nc, tc, y2d, e1t, d1t, e2t, d2t, out, gb, 4, eps_t)

    nc.compile()
    return nc


def _layernorm_store(nc, pool, rsd, dst_dram, gb, gbi, eps_t=None, also_bf16=False):
    """LN over the free axis of rsd [128, D] f32 (g/b optional), write f32
    tile to dst_dram; optionally return a bf16 copy of the normed tile."""
    st6 = pool.tile([128, 2, 6], f32, tag="ln_st6")
    mv = pool.tile([128, 2], f32, tag="ln_mv")
    nc.vector.bn_stats(st6[:, 0, :], rsd[:, 0:512])
    nc.vector.bn_stats(st6[:, 1, :], rsd[:, 512:1024])
    nc.vector.bn_aggr(mv[:], st6[:])
    sd = pool.tile([128, 1], f32, tag="ln_sd")
    nc.scalar.activation(sd[:], mv[:, 1:2], AF.Sqrt, bias=eps_t[:])
    rstd = pool.tile([128, 1], f32, tag="ln_rstd")
    nc.vector.reciprocal(rstd[:], sd[:])
    cneg = pool.tile([128, 1], f32, tag="ln_cneg")
    nc.vector.scalar_tensor_tensor(
        out=cneg[:], in0=mv[:, 0:1], scalar=-1.0, in1=rstd[:],
        op0=ALU.mult, op1=ALU.mult)
    yt = pool.tile([128, D], f32, tag="ln_out")
    nc.scalar.activation(yt[:], rsd[:], AF.Identity, scale=rstd[:], bias=cneg[:])
    if gb is not None:
        g_t, b_t = gb[gbi], gb[gbi + 1]
        nc.vector.tensor_tensor(out=yt[:], in0=yt[:], in1=g_t[:], op=ALU.mult)
        nc.vector.tensor_tensor(out=yt[:], in0=yt[:], in1=b_t[:], op=ALU.add)
    nc.sync.dma_start(dst_dram, yt[:])
    if also_bf16:
        yb = pool.tile([128, D], bf16, tag="ln_out_bf")
        nc.vector.tensor_copy(yb[:], yt[:])
        return yb
    return None


def _phase_attn(nc, tc, b_iter, masked, xq_nat, xqT_dram, kvT_dram,
                wqkv, wot, n_kv, maskt, y_next_d, gb, gbi, eps_t=None):
    """One attention phase (self or cross) for all batch elems."""
    with tc.tile_pool(name="attn_sb", bufs=1) as sb:
        # weights resident: wqkv rhs tiles [hg][k] for q/k/v + wot chunks
        wq_s, wk_s, wv_s = ([[None] * 8 for _ in range(2)] for _ in range(3))
        for hg in range(2):
            for k in range(8):
                for pi, ws in ((0, wq_s), (1, wk_s), (2, wv_s)):
                    t = sb.tile([128, 512], bf16, tag=f"w{pi}_{hg}_{k}")
                    nc.sync.dma_start(t[:], wqkv[pi, hg, k])
                    ws[hg][k] = t
        wot_s = []
        for j in range(8):
            t = sb.tile([128, D], bf16, tag=f"wot{j}")
            wot_s.append(t)
        wot_loaded = [False]

        for b in b_iter:
            # xqT tiles (lhsT for Q proj, and for self-attn also K/V)
            xqT = []
            if xqT_dram is not None:
                for k in range(8):
                    t = sb.tile([128, S_T], bf16, tag=f"xqT{k}")
                    nc.sync.dma_start(t[:], xqT_dram[b, k])
                    xqT.append(t)
            else:
                # rebuild transposed bf16 x from the natural f32 dram tensor
                for k in range(8):
                    xqT.append(sb.tile([128, S_T], bf16, tag=f"xqT{k}", name=f"xqT{k}"))
                for st in range(8):
                    nat = sb.tile([128, D], f32, tag="xq_nat_ld", bufs=2)
                    nc.sync.dma_start(nat[:], xq_nat[b, 128 * st:128 * (st + 1), :])
                    natb = sb.tile([128, D], bf16, tag="xq_nat_bf", bufs=2)
                    nc.vector.tensor_copy(natb[:], nat[:])
                    for k in range(8):
                        nc.sync.dma_start_transpose(
                            xqT[k][:, 128 * st:128 * (st + 1)],
                            natb[:, 128 * k:128 * (k + 1)])

            for hg in range(2):
                # ---- stage A: K/V projections + evac + A accumulation ----
                expk = sb.tile([128, n_kv, 512], bf16, tag="expk")
                expv = sb.tile([128, n_kv, 512], bf16, tag="expv")
                with tc.tile_pool(name="ps_a", bufs=1, space="PSUM") as psa:
                    for sm in range(n_kv):
                        kps = psa.tile([128, 512], f32, tag="kv", bufs=6)
                        vps = psa.tile([128, 512], f32, tag="kv", bufs=6)
                        for k in range(8):
                            if kvT_dram is None:
                                lhsT = xqT[k][:, 128 * sm:128 * (sm + 1)]
                            else:
                                lt = sb.tile([128, 128], bf16, tag="memlhs", bufs=4)
                                nc.sync.dma_start(lt[:], kvT_dram[b, k, sm])
                                lhsT = lt[:]
                            nc.tensor.matmul(kps[:], lhsT, wk_s[hg][k][:],
                                             start=(k == 0), stop=(k == 7))
                            nc.tensor.matmul(vps[:], lhsT, wv_s[hg][k][:],
                                             start=(k == 0), stop=(k == 7))
                        # evac: expK (bf16) + per-head rowsums; V' = V/rowsumK
                        nc.scalar.activation(
                            expk[:, sm, :], kps[:], AF.Exp, scale=1.0 / SCALE)
                        krs = sb.tile([128, 4], f32, tag="krs", bufs=2)
                        nc.vector.tensor_reduce(
                            out=krs[:],
                            in_=expk[:, sm, :].rearrange("p (h q) -> p h q", h=4),
                            axis=mybir.AxisListType.X, op=ALU.add)
                        krr = sb.tile([128, 4], f32, tag="krr", bufs=2)
                        nc.vector.reciprocal(krr[:], krs[:])
                        nc.vector.tensor_tensor(
                            out=expv[:, sm, :].rearrange("p (h q) -> p h q", h=4),
                            in0=vps[:].rearrange("p (h q) -> p h q", h=4),
                            in1=krr[:].unsqueeze(2).broadcast_to([128, 4, 128]),
                            op=ALU.mult)
                    # A for the 4 heads of this hg, packed in one psum bank
                    aps = psa.tile([128, 512], f32, tag="aps", bufs=2)
                    for hi in range(4):
                        for sm in range(n_kv):
                            nc.tensor.matmul(
                                aps[:, 128 * hi:128 * (hi + 1)],
                                expk[:, sm, 128 * hi:128 * (hi + 1)],
                                expv[:, sm, 128 * hi:128 * (hi + 1)],
                                start=(sm == 0), stop=(sm == n_kv - 1))
                    asb = sb.tile([128, 512], bf16, tag="asb")
                    nc.vector.tensor_copy(asb[:], aps[:])

                # ---- stage B: Q proj + softmax + transpose ----
                softqT = sb.tile([128, 4, S_T], bf16, tag="softqT")
                with tc.tile_pool(name="ps_b", bufs=1, space="PSUM") as psb:
                    for st in range(8):
                        qps = psb.tile([128, 512], f32, tag="qps", bufs=2)
                        for k in range(8):
                            nc.tensor.matmul(
                                qps[:], xqT[k][:, 128 * st:128 * (st + 1)],
                                wq_s[hg][k][:], start=(k == 0), stop=(k == 7))
                        if masked and st == 0:
                            nc.vector.tensor_tensor(
                                out=qps[:], in0=qps[:], in1=maskt[:], op=ALU.add)
                        eq = sb.tile([128, 512], f32, tag="eq", bufs=2)
                        nc.scalar.activation(eq[:], qps[:], AF.Exp, scale=1.0 / SCALE)
                        qrs = sb.tile([128, 4], f32, tag="qrs", bufs=2)
                        nc.vector.tensor_reduce(
                            out=qrs[:], in_=eq[:].rearrange("p (h q) -> p h q", h=4),
                            axis=mybir.AxisListType.X, op=ALU.add)
                        qrr = sb.tile([128, 4], f32, tag="qrr", bufs=2)
                        nc.vector.reciprocal(qrr[:], qrs[:])
                        sq = sb.tile([128, 4, 128], bf16, tag="sq", bufs=2)
                        nc.vector.tensor_tensor(
                            out=sq[:], in0=eq[:].rearrange("p (h q) -> p h q", h=4),
                            in1=qrr[:].unsqueeze(2).broadcast_to([128, 4, 128]),
                            op=ALU.mult)
                        for hi in range(4):
                            eng = nc.scalar if hi % 2 else nc.sync
                            eng.dma_start_transpose(
                                softqT[:, hi, 128 * st:128 * (st + 1)],
                                sq[:, hi, :])

                    # ---- stage C: Bm, Wo, residual + LN per head ----
                    if not wot_loaded[0]:
                        wot_loaded[0] = True
                        for j in range(8):
                            nc.sync.dma_start(wot_s[j][:], wot[j])
                    for hi in range(4):
                        hb = 4 * hg + hi  # head == output s-tile block
                        bmt = psb.tile([128, S_T], f32, tag="bmt")
                        nc.tensor.matmul(bmt[:, 0:512], asb[:, 128 * hi:128 * (hi + 1)],
                                         softqT[:, hi, 0:512])
                        nc.tensor.matmul(bmt[:, 512:1024], asb[:, 128 * hi:128 * (hi + 1)],
                                         softqT[:, hi, 512:1024])
                        bms = sb.tile([128, S_T], bf16, tag="bms", bufs=2)
                        nc.vector.tensor_copy(bms[:], bmt[:])
                        ops = psb.tile([128, D], f32, tag="ops", bufs=2)
                        for j in range(8):
                            for nh in range(2):
                                nc.tensor.matmul(
                                    ops[:, 512 * nh:512 * (nh + 1)],
                                    bms[:, j::8],
                                    wot_s[j][:, 512 * nh:512 * (nh + 1)],
                                    start=(j == 0), stop=(j == 7))
                        nat = sb.tile([128, D], f32, tag="res_nat", bufs=2)
                        nc.sync.dma_start(nat[:], xq_nat[b, 128 * hb:128 * (hb + 1), :])
                        rsd = sb.tile([128, D], f32, tag="rsd", bufs=2)
                        nc.vector.tensor_tensor(out=rsd[:], in0=ops[:], in1=nat[:],
                                                op=ALU.add)
                        _layernorm_store(
                            nc, sb, rsd, y_next_d[b, 128 * hb:128 * (hb + 1), :],
                            gb, gbi, eps_t)


def _phase_lffn(nc, tc, y2d, e1t, d1t, e2t, d2t, out, gb, gbi, eps_t=None):
    with tc.tile_pool(name="ffn_sb", bufs=1) as sb:
        e1s = [[None] * 4 for _ in range(8)]
        d1s = [[None] * 8 for _ in range(4)]
        e2s = [[None] * 4 for _ in range(8)]
        d2s = []
        for k in range(8):
            for t_ in range(4):
                e1s[k][t_] = sb.tile([128, 128], bf16, tag=f"e1_{k}_{t_}", name=f"e1_{k}_{t_}")
                nc.sync.dma_start(e1s[k][t_][:], e1t[k, t_])
                e2s[k][t_] = sb.tile([128, 128], bf16, tag=f"e2_{k}_{t_}", name=f"e2_{k}_{t_}")
                nc.sync.dma_start(e2s[k][t_][:], e2t[k, t_])
        for k in range(4):
            for t_ in range(8):
                d1s[k][t_] = sb.tile([128, 128], bf16, tag=f"d1_{k}_{t_}", name=f"d1_{k}_{t_}")
                nc.sync.dma_start(d1s[k][t_][:], d1t[k, t_])
            t = sb.tile([128, D], bf16, tag=f"d2_{k}")
            nc.sync.dma_start(t[:], d2t[k])
            d2s.append(t)

        for b in range(BPC):
            # y2T bf16 tiles rebuilt from y2 dram
            y2T = [sb.tile([128, S_T], bf16, tag=f"y2T{k}", name=f"y2T{k}") for k in range(8)]
            for st in range(8):
                nat = sb.tile([128, D], f32, tag="y2_nat_ld", bufs=2)
                nc.sync.dma_start(nat[:], y2d[b, 128 * st:128 * (st + 1), :])
                natb = sb.tile([128, D], bf16, tag="y2_nat_bf", bufs=2)
                nc.vector.tensor_copy(natb[:], nat[:])
                for k in range(8):
                    nc.sync.dma_start_transpose(
                        y2T[k][:, 128 * st:128 * (st + 1)],
                        natb[:, 128 * k:128 * (k + 1)])

            # h1T = E1 @ y2T  [BN(4 tiles), S_T]
            h1T = [sb.tile([128, S_T], bf16, tag=f"h1T{t_}", name=f"h1T{t_}") for t_ in range(4)]
            with tc.tile_pool(name="ps_f1", bufs=1, space="PSUM") as ps:
                for t_ in range(4):
                    acc = ps.tile([128, S_T], f32, tag="acc", bufs=3)
                    for nh in range(2):
                        for k in range(8):
                            nc.tensor.matmul(
                                acc[:, 512 * nh:512 * (nh + 1)], e1s[k][t_][:],
                                y2T[k][:, 512 * nh:512 * (nh + 1)],
                                start=(k == 0), stop=(k == 7))
                    nc.vector.tensor_copy(h1T[t_][:], acc[:])
            # h2T = D1 @ h1T -> silu -> swT  [HID(8 tiles), S_T]
            swT = [sb.tile([128, S_T], bf16, tag=f"swT{t_}", name=f"swT{t_}") for t_ in range(8)]
            with tc.tile_pool(name="ps_f2", bufs=1, space="PSUM") as ps:
                for t_ in range(8):
                    acc = ps.tile([128, S_T], f32, tag="acc", bufs=3)
                    for nh in range(2):
                        for k in range(4):
                            nc.tensor.matmul(
                                acc[:, 512 * nh:512 * (nh + 1)], d1s[k][t_][:],
                                h1T[k][:, 512 * nh:512 * (nh + 1)],
                                start=(k == 0), stop=(k == 3))
                    nc.scalar.activation(swT[t_][:], acc[:], AF.Silu)
            # g1T = E2 @ swT  [BN(4 tiles), S_T]
            g1T = [sb.tile([128, S_T], bf16, tag=f"g1T{t_}", name=f"g1T{t_}") for t_ in range(4)]
            with tc.tile_pool(name="ps_f3", bufs=1, space="PSUM") as ps:
                for t_ in range(4):
                    acc = ps.tile([128, S_T], f32, tag="acc", bufs=3)
                    for nh in range(2):
                        for k in range(8):
                            nc.tensor.matmul(
                                acc[:, 512 * nh:512 * (nh + 1)], e2s[k][t_][:],
                                swT[k][:, 512 * nh:512 * (nh + 1)],
                                start=(k == 0), stop=(k == 7))
                    nc.vector.tensor_copy(g1T[t_][:], acc[:])
            # ffn[st] = g1T[:, st].T @ D2T ; residual with y2, LN3 -> out
            with tc.tile_pool(name="ps_f4", bufs=1, space="PSUM") as ps:
                for st in range(8):
                    acc = ps.tile([128, D], f32, tag="acc", bufs=3)
                    for nh in range(2):
                        for k in range(4):
                            nc.tensor.matmul(
                                acc[:, 512 * nh:512 * (nh + 1)],
                                g1T[k][:, 128 * st:128 * (st + 1)],
                                d2s[k][:, 512 * nh:512 * (nh + 1)],
                                start=(k == 0), stop=(k == 3))
                    nat = sb.tile([128, D], f32, tag="y2res", bufs=2)
                    nc.sync.dma_start(nat[:], y2d[b, 128 * st:128 * (st + 1), :])
                    rsd = sb.tile([128, D], f32, tag="rsd", bufs=2)
                    nc.vector.tensor_tensor(out=rsd[:], in0=acc[:], in1=nat[:],
                                            op=ALU.add)
                    _layernorm_store(nc, sb, rsd,
                                     out[b, 128 * st:128 * (st + 1), :], gb, gbi,
                                     eps_t)


_CACHE = {}


def _prep_host(inputs):
    """Convert/transpose/tile weights + activations per the kernel layout."""
    g = {k: np.asarray(v) for k, v in inputs.items()}
    affine = not (
        np.all(g["g1"] == 1) and np.all(g["g2"] == 1) and np.all(g["g3"] == 1)
        and np.all(g["b1"] == 0) and np.all(g["b2"] == 0) and np.all(g["b3"] == 0))

    def wqkv_pack(q, k, v):
        # [H, D, DQ] -> [3][hg=2][kchunk=8][128, 512] (4 heads concat)
        def onev2(w):
            arr = np.empty((2, 8, 128, 512), np.float32)
            for hg in range(2):
                for kc in range(8):
                    cols = [w[4 * hg + hi, 128 * kc:128 * (kc + 1), :] for hi in range(4)]
                    arr[hg, kc] = np.concatenate(cols, axis=1)
            return arr
        return np.stack([onev2(q), onev2(k), onev2(v)]).astype(bf)

    host = {}
    host["wqkv1"] = wqkv_pack(g["Wq1"], g["Wk1"], g["Wv1"])
    host["wqkv2"] = wqkv_pack(g["Wq2"], g["Wk2"], g["Wv2"])
    host["wo1t"] = np.ascontiguousarray(g["Wo1"].T).reshape(8, 128, D).astype(bf)
    host["wo2t"] = np.ascontiguousarray(g["Wo2"].T).reshape(8, 128, D).astype(bf)
    host["e1t"] = np.ascontiguousarray(
        g["E1"].T).reshape(8, 128, 4, 128).transpose(0, 2, 1, 3).astype(bf)
    host["d1t"] = np.ascontiguousarray(
        g["D1"].T).reshape(4, 128, 8, 128).transpose(0, 2, 1, 3).astype(bf)
    host["e2t"] = np.ascontiguousarray(
        g["E2"].T).reshape(8, 128, 4, 128).transpose(0, 2, 1, 3).astype(bf)
    host["d2t"] = np.ascontiguousarray(g["D2"].T).reshape(4, 128, D).astype(bf)
    mask = np.where(np.arange(DQ)[None, :] <= np.arange(128)[:, None],
                    0.0, NEG).astype(np.float32)
    host["mask4"] = np.tile(mask, (1, 4))
    if affine:
        host["grep"] = np.stack([
            np.broadcast_to(g[n].astype(np.float32), (128, D))
            for n in ("g1", "b1", "g2", "b2", "g3", "b3")]).copy()

    in_maps = []
    y = g["y"].astype(np.float32)
    mem = g["mem"].astype(np.float32)
    for c in range(N_CORES):
        sl = slice(BPC * c, BPC * (c + 1))
        m = dict(host)
        m["y0"] = np.ascontiguousarray(y[sl])
        yT = np.ascontiguousarray(y[sl].transpose(0, 2, 1)).astype(bf)
        m["y0T"] = np.ascontiguousarray(yT.reshape(BPC, 8, 128, S_T))
        mT = np.ascontiguousarray(mem[sl].transpose(0, 2, 1)).astype(bf)
        m["memT"] = np.ascontiguousarray(
            mT.reshape(BPC, 8, 128, 16, 128).transpose(0, 1, 3, 2, 4))
        in_maps.append(m)
    return in_maps, affine


def kernel(**inputs):
    in_maps, affine = _prep_host(inputs)
    if affine not in _CACHE:
        _CACHE[affine] = _build(affine)
    nc = _CACHE[affine]
    res = run_bass_kernel_spmd(nc, in_maps, list(range(N_CORES)))
    return np.concatenate([r["out"] for r in res.results], axis=0)


if __name__ == "__main__":
    rng = np.random.default_rng(0)
    ins = {
        "mem": rng.standard_normal((B, S_M, D), dtype=np.float32),
        "y": rng.standard_normal((B, S_T, D), dtype=np.float32),
        **{k: (rng.standard_normal(s, dtype=np.float32) * 0.02).astype(np.float32)
           for k, s in {
               "Wq1": (H, D, DQ), "Wk1": (H, D, DQ), "Wv1": (H, D, DQ),
               "Wo1": (D, D), "Wq2": (H, D, DQ), "Wk2": (H, D, DQ),
               "Wv2": (H, D, DQ), "Wo2": (D, D), "E1": (BNK, D),
               "D1": (HID, BNK), "E2": (BNK, HID), "D2": (D, BNK)}.items()},
        "g1": np.ones(D, np.float32), "b1": np.zeros(D, np.float32),
        "g2": np.ones(D, np.float32), "b2": np.zeros(D, np.float32),
        "g3": np.ones(D, np.float32), "b3": np.zeros(D, np.float32),
    }
    o = kernel(**ins)
    print("out", o.shape, o.dtype, np.abs(o).mean())



# revision 11
# speedup vs baseline: 1.6439x; 1.6439x over previous
# Trainium2 Bass kernel for nn_DecoderBlock (masked self-attn + cross-attn +
# LFFN decoder block with "linear" softmax attention over the head dim).
#
# Sharding: data-parallel over batch — 16 batch elems / 8 cores = 2 per core.
# All weights replicated per core (bf16); activations stream per batch elem.
#
# Math per core/batch elem (validated against the jax reference in numpy):
#   per head: Q/K/V = x @ W[h]        ([s, dq] layout, s on partitions)
#   expQ/expK = exp((Q|K)/DQ**0.25)   (mask added to Q rows < 127 first)
#   V' = V * (1/rowsum(expK))         (folds K-softmax denominator)
#   A  = expK^T @ V'                  ([dq, dq])
#   softQ = expQ * (1/rowsum(expQ));  softQT = PE-transpose(softQ)   [dq, s]
#   BmT = A^T @ softQT                ([dq, s])
#   out rows [128h:128h+128] = sum_j BmT[:, j::8].T @ Wo.T[128j:128j+128, :]
#     (replicates the module's raw [b,h,s,d] -> [b, s, h*d] view)
#   residual + layernorm in natural [s, D] layout; transposed copy of the LN
#   output is produced on the PE for the next phase's lhsT operands.
#
# All weights are host-packed into [128, ...] images so each group loads with
# ONE big DMA; all transposes run on the TensorE (identity matmul) instead of
# the descriptor-bound DMA-transpose path.
import numpy as np
import ml_dtypes

import concourse.bacc as bacc
import concourse.mybir as mybir
import concourse.tile as tile
from concourse.bass_utils import run_bass_kernel_spmd
from concourse.masks import make_identity

H, D, DQ, BNK, HID = 8, 1024, 128, 512, 1024
B, S_T, S_M = 16, 1024, 2048
SCALE = DQ ** 0.25
EPS = 1e-5
NEG = -200.0
N_CORES = 8
BPC = B // N_CORES  # batch elems per core

f32 = mybir.dt.float32
bf16 = mybir.dt.bfloat16
AF = mybir.ActivationFunctionType
ALU = mybir.AluOpType
bf = ml_dtypes.bfloat16


def _build(affine: bool):
    nc = bacc.Bacc("TRN2", target_bir_lowering=False, debug=False,
                   enable_asserts=True, num_devices=N_CORES)

    def din(name, shape, dt=bf16):
        return nc.dram_tensor(name, list(shape), dt, kind="ExternalInput").ap()

    y0b = din("y0b", [BPC, S_T, D])                  # natural bf16 (residual)
    y0T = din("y0T", [BPC, 8, 128, S_T])             # [b][kchunk][128, S_T]
    memTp = din("memTp", [BPC, 8, 128, 2, 8, 128])   # [b][jpair][p][i][k][q]
    wqkv1 = din("wqkv1", [128, 3, 2, 8, 512])        # [p][qkv][hg][kchunk][512]
    wqkv2 = din("wqkv2", [128, 3, 2, 8, 512])
    wo1t = din("wo1t", [128, 8, D])                  # [p][j][D]
    wo2t = din("wo2t", [128, 8, D])
    e1w = din("e1w", [128, 8, 4, 128])               # [p][kchunk][bn_tile][q]
    d1w = din("d1w", [128, 4, 8, 128])               # [p][bn_chunk][hid_tile][q]
    e2w = din("e2w", [128, 8, 4, 128])               # [p][hid_chunk][bn_tile][q]
    d2w = din("d2w", [128, 4, D])                    # [p][bn_chunk][D]
    mask4 = din("mask4", [128, 512], f32)
    grep = din("grep", [6, 128, D], f32) if affine else None

    out = nc.dram_tensor("out", [BPC, S_T, D], f32, kind="ExternalOutput").ap()

    with tile.TileContext(nc) as tc:
        with tc.tile_pool(name="dram", bufs=1, space="DRAM") as dpool:
            y1d = dpool.tile([BPC, S_T, D], bf16)
            y2d = dpool.tile([BPC, S_T, D], bf16)

            with tc.tile_pool(name="consts", bufs=1) as cpool:
                maskt = cpool.tile([128, 512], f32, tag="maskt")
                nc.sync.dma_start(maskt[:], mask4[:])
                eps_t = cpool.tile([128, 1], f32, tag="eps_t")
                nc.vector.memset(eps_t[:], EPS)
                ident = cpool.tile([128, 128], bf16, tag="ident")
                make_identity(nc, ident[:])
                gb = None
                if affine:
                    gb = [cpool.tile([128, D], f32, tag=f"gb{i}", name=f"gb{i}")
                          for i in range(6)]
                    for i in range(6):
                        nc.sync.dma_start(gb[i][:], grep[i])

                # persistent transposed-activation pool: 8 k-chunk tags,
                # 3 rotating buffers each (y1T b0, y1T b1, y2T b0 reuses...)
                with tc.tile_pool(name="xT", bufs=1) as xpool:
                    def xt_alloc():
                        return [xpool.tile([128, S_T], bf16, tag=f"xT{k}",
                                           name=f"xT{k}", bufs=3)
                                for k in range(8)]

                    y1T = [None] * BPC
                    y2T = [None] * BPC
                    ctx = dict(nc=nc, tc=tc, maskt=maskt, eps_t=eps_t,
                               ident=ident, gb=gb)

                    _phase_attn(ctx, masked=True, xq_dram=y0T, memT=None,
                                wqkv=wqkv1, wot=wo1t, res_d=y0b,
                                y_next_d=y1d, xT_in=None, xT_out=y1T,
                                xt_alloc=xt_alloc, gbi=0)
                    _phase_attn(ctx, masked=False, xq_dram=None, memT=memTp,
                                wqkv=wqkv2, wot=wo2t, res_d=y1d,
                                y_next_d=y2d, xT_in=y1T, xT_out=y2T,
                                xt_alloc=xt_alloc, gbi=2)
                    _phase_lffn(ctx, y2T, e1w, d1w, e2w, d2w, y2d, out, gbi=4)

    nc.compile()
    return nc


def _layernorm(ctx, pool, rsd, dst_dram, gbi, out_dt):
    """LN over the free axis of rsd [128, D] f32; write `out_dt` tile to
    dst_dram and return the SBUF tile."""
    nc, eps_t, gb = ctx["nc"], ctx["eps_t"], ctx["gb"]
    st6 = pool.tile([128, 2, 6], f32, tag="ln_st6", bufs=2)
    mv = pool.tile([128, 2], f32, tag="ln_mv", bufs=2)
    nc.vector.bn_stats(st6[:, 0, :], rsd[:, 0:512])
    nc.vector.bn_stats(st6[:, 1, :], rsd[:, 512:1024])
    nc.vector.bn_aggr(mv[:], st6[:])
    sd = pool.tile([128, 1], f32, tag="ln_sd", bufs=2)
    nc.scalar.activation(sd[:], mv[:, 1:2], AF.Sqrt, bias=eps_t[:])
    rstd = pool.tile([128, 1], f32, tag="ln_rstd", bufs=2)
    nc.vector.reciprocal(rstd[:], sd[:])
    cneg = pool.tile([128, 1], f32, tag="ln_cneg", bufs=2)
    nc.vector.scalar_tensor_tensor(
        out=cneg[:], in0=mv[:, 0:1], scalar=-1.0, in1=rstd[:],
        op0=ALU.mult, op1=ALU.mult)
    yt = pool.tile([128, D], out_dt, tag="ln_out", bufs=2)
    nc.scalar.activation(yt[:], rsd[:], AF.Identity, scale=rstd[:], bias=cneg[:])
    if gb is not None:
        g_t, b_t = gb[gbi], gb[gbi + 1]
        nc.vector.tensor_tensor(out=yt[:], in0=yt[:], in1=g_t[:], op=ALU.mult)
        nc.vector.tensor_tensor(out=yt[:], in0=yt[:], in1=b_t[:], op=ALU.add)
    nc.sync.dma_start(dst_dram, yt[:])
    return yt


def _phase_attn(ctx, masked, xq_dram, memT, wqkv, wot, res_d, y_next_d,
                xT_in, xT_out, xt_alloc, gbi):
    """One attention phase (self or cross) for all batch elems."""
    nc, tc, ident = ctx["nc"], ctx["tc"], ctx["ident"]
    n_kv = 8 if memT is None else 16
    with tc.tile_pool(name="attn_sb", bufs=1) as sb:
        w = sb.tile([128, 3, 2, 8, 512], bf16, tag="w")
        nc.sync.dma_start(w[:], wqkv[:])
        wo = sb.tile([128, 8, D], bf16, tag="wo")
        nc.sync.dma_start(wo[:], wot[:])

        with tc.tile_pool(name="attn_ps", bufs=1, space="PSUM") as ps:
            for b in range(BPC):
                # ---- lhsT sources ----
                if xq_dram is not None:
                    xqT = [sb.tile([128, S_T], bf16, tag=f"xqT{k}",
                                   name=f"xqT{k}") for k in range(8)]
                    for k in range(8):
                        nc.sync.dma_start(xqT[k][:], xq_dram[b, k])
                else:
                    xqT = xT_in[b]

                xt_next = xt_alloc()
                xT_out[b] = xt_next
                for hg in range(2):
                    # ---- stage A: K/V proj + exp/evac + A accumulation ----
                    expk = sb.tile([128, n_kv, 512], bf16, tag="expk")
                    expv = sb.tile([128, n_kv, 512], bf16, tag="expv")
                    for j in range(n_kv // 2):
                        if memT is not None:
                            mt = sb.tile([128, 2, 8, 128], bf16, tag="mt",
                                         bufs=3)
                            nc.sync.dma_start(mt[:], memT[b, j])
                        for i in range(2):
                            sm = 2 * j + i
                            kps = ps.tile([128, 512], f32, tag="ps512", bufs=3)
                            vps = ps.tile([128, 512], f32, tag="ps512", bufs=3)
                            for k in range(8):
                                if memT is None:
                                    lhsT = xqT[k][:, 128 * sm:128 * (sm + 1)]
                                else:
                                    lhsT = mt[:, i, k, :]
                                nc.tensor.matmul(kps[:], lhsT, w[:, 1, hg, k, :],
                                                 start=(k == 0), stop=(k == 7))
                                nc.tensor.matmul(vps[:], lhsT, w[:, 2, hg, k, :],
                                                 start=(k == 0), stop=(k == 7))
                            nc.scalar.activation(expk[:, sm, :], kps[:], AF.Exp,
                                                 scale=1.0 / SCALE)
                            krs = sb.tile([128, 4], f32, tag="krs", bufs=2)
                            nc.vector.tensor_reduce(
                                out=krs[:],
                                in_=expk[:, sm, :].rearrange("p (h q) -> p h q", h=4),
                                axis=mybir.AxisListType.X, op=ALU.add)
                            krr = sb.tile([128, 4], f32, tag="krr", bufs=2)
                            nc.vector.reciprocal(krr[:], krs[:])
                            nc.vector.tensor_tensor(
                                out=expv[:, sm, :].rearrange("p (h q) -> p h q", h=4),
                                in0=vps[:].rearrange("p (h q) -> p h q", h=4),
                                in1=krr[:].unsqueeze(2).broadcast_to([128, 4, 128]),
                                op=ALU.mult)
                    # per-region accumulation groups must stay consecutive:
                    # interleaving groups within one PSUM bank corrupts them.
                    aps = ps.tile([128, 512], f32, tag="aps", bufs=2)
                    for hi in range(4):
                        for sm in range(n_kv):
                            nc.tensor.matmul(
                                aps[:, 128 * hi:128 * (hi + 1)],
                                expk[:, sm, 128 * hi:128 * (hi + 1)],
                                expv[:, sm, 128 * hi:128 * (hi + 1)],
                                start=(sm == 0), stop=(sm == n_kv - 1))
                    asb = sb.tile([128, 512], bf16, tag="asb", bufs=2)
                    nc.vector.tensor_copy(asb[:], aps[:])

                    # ---- stage B: Q proj + softmax + PE transpose ----
                    softqT = sb.tile([128, 4, S_T], bf16, tag="softqT", bufs=2)
                    for st in range(8):
                        qps = ps.tile([128, 512], f32, tag="ps512", bufs=3)
                        for k in range(8):
                            nc.tensor.matmul(
                                qps[:], xqT[k][:, 128 * st:128 * (st + 1)],
                                w[:, 0, hg, k, :], start=(k == 0), stop=(k == 7))
                        if masked and st == 0:
                            nc.vector.tensor_tensor(
                                out=qps[:], in0=qps[:], in1=ctx["maskt"][:],
                                op=ALU.add)
                        eq = sb.tile([128, 512], f32, tag="eq", bufs=2)
                        nc.scalar.activation(eq[:], qps[:], AF.Exp,
                                             scale=1.0 / SCALE)
                        qrs = sb.tile([128, 4], f32, tag="qrs", bufs=2)
                        nc.vector.tensor_reduce(
                            out=qrs[:], in_=eq[:].rearrange("p (h q) -> p h q", h=4),
                            axis=mybir.AxisListType.X, op=ALU.add)
                        qrr = sb.tile([128, 4], f32, tag="qrr", bufs=2)
                        nc.vector.reciprocal(qrr[:], qrs[:])
                        sq = sb.tile([128, 4, 128], bf16, tag="sq", bufs=2)
                        nc.vector.tensor_tensor(
                            out=sq[:], in0=eq[:].rearrange("p (h q) -> p h q", h=4),
                            in1=qrr[:].unsqueeze(2).broadcast_to([128, 4, 128]),
                            op=ALU.mult)
                        tp = ps.tile([128, 1024], bf16, tag="tpb", bufs=1)
                        for hi in range(4):
                            nc.tensor.transpose(
                                tp[:, 128 * hi:128 * (hi + 1)], sq[:, hi, :],
                                ident[:])
                        for hi in range(4):
                            nc.scalar.activation(
                                softqT[:, hi, 128 * st:128 * (st + 1)],
                                tp[:, 128 * hi:128 * (hi + 1)], AF.Identity)

                    # ---- stage C: Bm, Wo, residual + LN (+ xT) per head ----
                    for hi in range(4):
                        hb = 4 * hg + hi  # head == output s-tile block
                        nat = sb.tile([128, D], bf16, tag="res_nat", bufs=2)
                        nc.sync.dma_start(
                            nat[:], res_d[b, 128 * hb:128 * (hb + 1), :])
                        bms = sb.tile([128, S_T], bf16, tag="bms", bufs=2)
                        for half in range(2):
                            bmt = ps.tile([128, 512], f32, tag="ps512", bufs=3)
                            nc.tensor.matmul(bmt[:],
                                             asb[:, 128 * hi:128 * (hi + 1)],
                                             softqT[:, hi,
                                                    512 * half:512 * (half + 1)])
                            nc.vector.tensor_copy(
                                bms[:, 512 * half:512 * (half + 1)], bmt[:])
                        ops = ps.tile([128, D], f32, tag="ps1k", bufs=1)
                        for jj in range(8):
                            for nh in range(2):
                                nc.tensor.matmul(
                                    ops[:, 512 * nh:512 * (nh + 1)],
                                    bms[:, jj::8],
                                    wo[:, jj, 512 * nh:512 * (nh + 1)],
                                    start=(jj == 0), stop=(jj == 7))
                        rsd = sb.tile([128, D], f32, tag="rsd", bufs=2)
                        nc.vector.tensor_tensor(out=rsd[:], in0=ops[:],
                                                in1=nat[:], op=ALU.add)
                        yb = _layernorm(ctx, sb,  rsd,
                                        y_next_d[b, 128 * hb:128 * (hb + 1), :],
                                        gbi, bf16)
                        tp2 = ps.tile([128, 1024], bf16, tag="tpb", bufs=1)
                        for k in range(8):
                            nc.tensor.transpose(
                                tp2[:, 128 * k:128 * (k + 1)],
                                yb[:, 128 * k:128 * (k + 1)], ident[:])
                        for k in range(8):
                            nc.scalar.activation(
                                xt_next[k][:, 128 * hb:128 * (hb + 1)],
                                tp2[:, 128 * k:128 * (k + 1)], AF.Identity)


def _phase_lffn(ctx, y2T, e1w_d, d1w_d, e2w_d, d2w_d, y2d, out, gbi):
    nc, tc = ctx["nc"], ctx["tc"]
    with tc.tile_pool(name="ffn_sb", bufs=1) as sb:
        e1 = sb.tile([128, 8, 4, 128], bf16, tag="e1")
        nc.sync.dma_start(e1[:], e1w_d[:])
        d1 = sb.tile([128, 4, 8, 128], bf16, tag="d1")
        nc.sync.dma_start(d1[:], d1w_d[:])
        e2 = sb.tile([128, 8, 4, 128], bf16, tag="e2")
        nc.sync.dma_start(e2[:], e2w_d[:])
        d2 = sb.tile([128, 4, D], bf16, tag="d2")
        nc.sync.dma_start(d2[:], d2w_d[:])

        with tc.tile_pool(name="ffn_ps", bufs=1, space="PSUM") as ps:
            for b in range(BPC):
                xT = y2T[b]
                # h1T = E1 @ y2T  [BN(4 tiles), S_T]
                h1T = [sb.tile([128, S_T], bf16, tag=f"h1T{t_}",
                               name=f"h1T{t_}") for t_ in range(4)]
                for t_ in range(4):
                    acc = ps.tile([128, S_T], f32, tag="acc", bufs=3)
                    for nh in range(2):
                        for k in range(8):
                            nc.tensor.matmul(
                                acc[:, 512 * nh:512 * (nh + 1)], e1[:, k, t_, :],
                                xT[k][:, 512 * nh:512 * (nh + 1)],
                                start=(k == 0), stop=(k == 7))
                    nc.vector.tensor_copy(h1T[t_][:], acc[:])
                # h2T = D1 @ h1T -> silu -> swT  [HID(8 tiles), S_T]
                swT = [sb.tile([128, S_T], bf16, tag=f"swT{t_}",
                               name=f"swT{t_}") for t_ in range(8)]
                for t_ in range(8):
                    acc = ps.tile([128, S_T], f32, tag="acc", bufs=3)
                    for nh in range(2):
                        for k in range(4):
                            nc.tensor.matmul(
                                acc[:, 512 * nh:512 * (nh + 1)], d1[:, k, t_, :],
                                h1T[k][:, 512 * nh:512 * (nh + 1)],
                                start=(k == 0), stop=(k == 3))
                    nc.scalar.activation(swT[t_][:], acc[:], AF.Silu)
                # g1T = E2 @ swT  [BN(4 tiles), S_T]
                g1T = [sb.tile([128, S_T], bf16, tag=f"g1T{t_}",
                               name=f"g1T{t_}") for t_ in range(4)]
                for t_ in range(4):
                    acc = ps.tile([128, S_T], f32, tag="acc", bufs=3)
                    for nh in range(2):
                        for k in range(8):
                            nc.tensor.matmul(
                                acc[:, 512 * nh:512 * (nh + 1)], e2[:, k, t_, :],
                                swT[k][:, 512 * nh:512 * (nh + 1)],
                                start=(k == 0), stop=(k == 7))
                    nc.vector.tensor_copy(g1T[t_][:], acc[:])
                # ffn[st] = g1T[:, st].T @ D2T ; residual with y2, LN3 -> out
                for st in range(8):
                    nat = sb.tile([128, D], bf16, tag="y2res", bufs=2)
                    nc.sync.dma_start(nat[:],
                                      y2d[b, 128 * st:128 * (st + 1), :])
                    acc = ps.tile([128, D], f32, tag="acc2", bufs=1)
                    for nh in range(2):
                        for k in range(4):
                            nc.tensor.matmul(
                                acc[:, 512 * nh:512 * (nh + 1)],
                                g1T[k][:, 128 * st:128 * (st + 1)],
                                d2[:, k, 512 * nh:512 * (nh + 1)],
                                start=(k == 0), stop=(k == 3))
                    rsd = sb.tile([128, D], f32, tag="rsd", bufs=2)
                    nc.vector.tensor_tensor(out=rsd[:], in0=acc[:], in1=nat[:],
                                            op=ALU.add)
                    _layernorm(ctx, sb, rsd,
                               out[b, 128 * st:128 * (st + 1), :], gbi, f32)


_CACHE = {}


def _prep_host(inputs):
    """Convert/transpose/pack weights + activations per the kernel layout."""
    g = {k: np.asarray(v) for k, v in inputs.items()}
    affine = not (
        np.all(g["g1"] == 1) and np.all(g["g2"] == 1) and np.all(g["g3"] == 1)
        and np.all(g["b1"] == 0) and np.all(g["b2"] == 0) and np.all(g["b3"] == 0))

    def wqkv_pack(q, k, v):
        # [H, D, DQ] -> [p=128][qkv][hg][kchunk][512] (4 heads concat)
        def onev2(w):
            arr = np.empty((2, 8, 128, 512), np.float32)
            for hg in range(2):
                for kc in range(8):
                    cols = [w[4 * hg + hi, 128 * kc:128 * (kc + 1), :]
                            for hi in range(4)]
                    arr[hg, kc] = np.concatenate(cols, axis=1)
            return arr
        st = np.stack([onev2(q), onev2(k), onev2(v)])  # [3,2,8,128,512]
        return np.ascontiguousarray(st.transpose(3, 0, 1, 2, 4)).astype(bf)

    host = {}
    host["wqkv1"] = wqkv_pack(g["Wq1"], g["Wk1"], g["Wv1"])
    host["wqkv2"] = wqkv_pack(g["Wq2"], g["Wk2"], g["Wv2"])
    host["wo1t"] = np.ascontiguousarray(
        g["Wo1"].T.reshape(8, 128, D).transpose(1, 0, 2)).astype(bf)
    host["wo2t"] = np.ascontiguousarray(
        g["Wo2"].T.reshape(8, 128, D).transpose(1, 0, 2)).astype(bf)
    host["e1w"] = np.ascontiguousarray(
        g["E1"].T.reshape(8, 128, 4, 128).transpose(1, 0, 2, 3)).astype(bf)
    host["d1w"] = np.ascontiguousarray(
        g["D1"].T.reshape(4, 128, 8, 128).transpose(1, 0, 2, 3)).astype(bf)
    host["e2w"] = np.ascontiguousarray(
        g["E2"].T.reshape(8, 128, 4, 128).transpose(1, 0, 2, 3)).astype(bf)
    host["d2w"] = np.ascontiguousarray(
        g["D2"].T.reshape(4, 128, D).transpose(1, 0, 2)).astype(bf)
    mask = np.where(np.arange(DQ)[None, :] <= np.arange(128)[:, None],
                    0.0, NEG).astype(np.float32)
    host["mask4"] = np.tile(mask, (1, 4))
    if affine:
        host["grep"] = np.stack([
            np.broadcast_to(g[n].astype(np.float32), (128, D))
            for n in ("g1", "b1", "g2", "b2", "g3", "b3")]).copy()

    in_maps = []
    y = g["y"].astype(np.float32)
    mem = g["mem"].astype(np.float32)
    for c in range(N_CORES):
        sl = slice(BPC * c, BPC * (c + 1))
        m = dict(host)
        m["y0b"] = y[sl].astype(bf)
        yT = np.ascontiguousarray(y[sl].transpose(0, 2, 1)).astype(bf)
        m["y0T"] = np.ascontiguousarray(yT.reshape(BPC, 8, 128, S_T))
        mT = mem[sl].transpose(0, 2, 1).astype(bf)  # [b, D, S_M]
        # [b, k, p, j, i, q] -> [b, j, p, i, k, q]
        m["memTp"] = np.ascontiguousarray(
            mT.reshape(BPC, 8, 128, 8, 2, 128).transpose(0, 3, 2, 4, 1, 5))
        in_maps.append(m)
    return in_maps, affine


def kernel(**inputs):
    in_maps, affine = _prep_host(inputs)
    if affine not in _CACHE:
        _CACHE[affine] = _build(affine)
    nc = _CACHE[affine]
    res = run_bass_kernel_spmd(nc, in_maps, list(range(N_CORES)))
    return np.concatenate([r["out"] for r in res.results], axis=0)


if __name__ == "__main__":
    rng = np.random.default_rng(0)
    ins = {
        "mem": rng.standard_normal((B, S_M, D), dtype=np.float32),
        "y": rng.standard_normal((B, S_T, D), dtype=np.float32),
        **{k: (rng.standard_normal(s, dtype=np.float32) * 0.02).astype(np.float32)
           for k, s in {
               "Wq1": (H, D, DQ), "Wk1": (H, D, DQ), "Wv1": (H, D, DQ),
               "Wo1": (D, D), "Wq2": (H, D, DQ), "Wk2": (H, D, DQ),
               "Wv2": (H, D, DQ), "Wo2": (D, D), "E1": (BNK, D),
               "D1": (HID, BNK), "E2": (BNK, HID), "D2": (D, BNK)}.items()},
        "g1": np.ones(D, np.float32), "b1": np.zeros(D, np.float32),
        "g2": np.ones(D, np.float32), "b2": np.zeros(D, np.float32),
        "g3": np.ones(D, np.float32), "b3": np.zeros(D, np.float32),
    }
    o = kernel(**ins)
    print("out", o.shape, o.dtype, np.abs(o).mean())


# revision 15
# speedup vs baseline: 1.7372x; 1.0567x over previous
# Trainium2 Bass kernel for nn_DecoderBlock (masked self-attn + cross-attn +
# LFFN decoder block with "linear" softmax attention over the head dim).
#
# Sharding: data-parallel over batch — 16 batch elems / 8 cores = 2 per core.
# All weights replicated per core (bf16); activations stream per batch elem.
#
# Math per core/batch elem (validated against the jax reference in numpy):
#   per head: Q/K/V = x @ W[h]        ([s, dq] layout, s on partitions)
#   expQ/expK = exp((Q|K)/DQ**0.25)   (mask added to Q rows < 127 first)
#   V' = V * (1/rowsum(expK))         (folds K-softmax denominator)
#   A  = expK^T @ V'                  ([dq, dq])
#   softQ = expQ * (1/rowsum(expQ));  softQT = PE-transpose(softQ)   [dq, s]
#   BmT = A^T @ softQT                ([dq, s])
#   out rows [128h:128h+128] = sum_j BmT[:, j::8].T @ Wo.T[128j:128j+128, :]
#     (replicates the module's raw [b,h,s,d] -> [b, s, h*d] view)
#   residual + layernorm in natural [s, D] layout; transposed copy of the LN
#   output is produced on the PE for the next phase's lhsT operands.
#
# All weights are host-packed into [128, ...] images so each group loads with
# ONE big DMA; all transposes run on the TensorE (identity matmul) instead of
# the descriptor-bound DMA-transpose path.
import numpy as np
import ml_dtypes

import concourse.bacc as bacc
import concourse.mybir as mybir
import concourse.tile as tile
from concourse.bass_utils import run_bass_kernel_spmd
from concourse.masks import make_identity

H, D, DQ, BNK, HID = 8, 1024, 128, 512, 1024
B, S_T, S_M = 16, 1024, 2048
SCALE = DQ ** 0.25
EPS = 1e-5
NEG = -200.0
N_CORES = 8
BPC = B // N_CORES  # batch elems per core

f32 = mybir.dt.float32
bf16 = mybir.dt.bfloat16
AF = mybir.ActivationFunctionType
ALU = mybir.AluOpType
bf = ml_dtypes.bfloat16


def _build(affine: bool):
    nc = bacc.Bacc("TRN2", target_bir_lowering=False, debug=False,
                   enable_asserts=True, num_devices=N_CORES)

    def din(name, shape, dt=bf16):
        return nc.dram_tensor(name, list(shape), dt, kind="ExternalInput").ap()

    y0b = din("y0b", [BPC, S_T, D])                  # natural bf16 (residual)
    y0T = din("y0T", [BPC, 128, 8, S_T])             # [b][128][kchunk][S_T]
    memTp = din("memTp", [BPC, 8, 128, 2, 8, 128])   # [b][jpair][p][i][k][q]
    wqkv1 = din("wqkv1", [128, 3, 2, 8, 512])        # [p][qkv][hg][kchunk][512]
    wqkv2 = din("wqkv2", [128, 3, 2, 8, 512])
    wo1t = din("wo1t", [128, 8, D])                  # [p][j][D]
    wo2t = din("wo2t", [128, 8, D])
    e1w = din("e1w", [128, 8, 4, 128])               # [p][kchunk][bn_tile][q]
    d1w = din("d1w", [128, 4, 8, 128])               # [p][bn_chunk][hid_tile][q]
    e2w = din("e2w", [128, 8, 4, 128])               # [p][hid_chunk][bn_tile][q]
    d2w = din("d2w", [128, 4, D])                    # [p][bn_chunk][D]
    mask4 = din("mask4", [128, 512], f32)
    grep = din("grep", [6, 128, D], f32) if affine else None

    out = nc.dram_tensor("out", [BPC, S_T, D], f32, kind="ExternalOutput").ap()

    with tile.TileContext(nc) as tc:
        with tc.tile_pool(name="dram", bufs=1, space="DRAM") as dpool:
            y1d = dpool.tile([BPC, S_T, D], bf16)
            y2d = dpool.tile([BPC, S_T, D], bf16)

            with tc.tile_pool(name="consts", bufs=1) as cpool:
                maskt = cpool.tile([128, 512], f32, tag="maskt")
                nc.sync.dma_start(maskt[:], mask4[:])
                eps_t = cpool.tile([128, 1], f32, tag="eps_t")
                nc.vector.memset(eps_t[:], EPS)
                ident = cpool.tile([128, 128], bf16, tag="ident")
                make_identity(nc, ident[:])
                gb = None
                if affine:
                    gb = [cpool.tile([128, D], f32, tag=f"gb{i}", name=f"gb{i}")
                          for i in range(6)]
                    for i in range(6):
                        nc.sync.dma_start(gb[i][:], grep[i])

                # persistent transposed-activation pool: one [128, 8, S_T]
                # tile per generation, 3 rotating buffers (y1T b0, y1T b1,
                # y2T b0; y2T b1 reuses y1T b0's buffer after last read)
                with tc.tile_pool(name="xT", bufs=1) as xpool:
                    def xt_alloc():
                        return xpool.tile([128, 8, S_T], bf16, tag="xT",
                                          name="xT", bufs=3)

                    y1T = [None] * BPC
                    y2T = [None] * BPC
                    ctx = dict(nc=nc, tc=tc, maskt=maskt, eps_t=eps_t,
                               ident=ident, gb=gb)

                    _phase_attn(ctx, masked=True, xq_dram=y0T, memT=None,
                                wqkv=wqkv1, wot=wo1t, res_d=y0b,
                                y_next_d=y1d, xT_in=None, xT_out=y1T,
                                xt_alloc=xt_alloc, gbi=0)
                    _phase_attn(ctx, masked=False, xq_dram=None, memT=memTp,
                                wqkv=wqkv2, wot=wo2t, res_d=y1d,
                                y_next_d=y2d, xT_in=y1T, xT_out=y2T,
                                xt_alloc=xt_alloc, gbi=2)
                    _phase_lffn(ctx, y2T, e1w, d1w, e2w, d2w, y2d, out, gbi=4)

    nc.compile()
    return nc


def _layernorm(ctx, pool, rsd, dst_dram, gbi, out_dt):
    """LN over the free axis of rsd [128, D] f32; write `out_dt` tile to
    dst_dram and return the SBUF tile."""
    nc, eps_t, gb = ctx["nc"], ctx["eps_t"], ctx["gb"]
    st6 = pool.tile([128, 2, 6], f32, tag="ln_st6", bufs=2)
    mv = pool.tile([128, 2], f32, tag="ln_mv", bufs=2)
    nc.vector.bn_stats(st6[:, 0, :], rsd[:, 0:512])
    nc.vector.bn_stats(st6[:, 1, :], rsd[:, 512:1024])
    nc.vector.bn_aggr(mv[:], st6[:])
    sd = pool.tile([128, 1], f32, tag="ln_sd", bufs=2)
    nc.scalar.activation(sd[:], mv[:, 1:2], AF.Sqrt, bias=eps_t[:])
    rstd = pool.tile([128, 1], f32, tag="ln_rstd", bufs=2)
    nc.vector.reciprocal(rstd[:], sd[:])
    cneg = pool.tile([128, 1], f32, tag="ln_cneg", bufs=2)
    nc.vector.scalar_tensor_tensor(
        out=cneg[:], in0=mv[:, 0:1], scalar=-1.0, in1=rstd[:],
        op0=ALU.mult, op1=ALU.mult)
    yt = pool.tile([128, D], out_dt, tag="ln_out", bufs=4)
    nc.scalar.activation(yt[:], rsd[:], AF.Identity, scale=rstd[:], bias=cneg[:])
    if gb is not None:
        g_t, b_t = gb[gbi], gb[gbi + 1]
        nc.vector.tensor_tensor(out=yt[:], in0=yt[:], in1=g_t[:], op=ALU.mult)
        nc.vector.tensor_tensor(out=yt[:], in0=yt[:], in1=b_t[:], op=ALU.add)
    nc.sync.dma_start(dst_dram, yt[:])
    return yt


def _phase_attn(ctx, masked, xq_dram, memT, wqkv, wot, res_d, y_next_d,
                xT_in, xT_out, xt_alloc, gbi):
    """One attention phase (self or cross) for all batch elems.

    Weights load on the sync HWDGE ring (K/V slab first so stage A can start
    early); activations/residuals use the scalar ring so the two FIFOs don't
    serialize each other.
    """
    nc, tc, ident = ctx["nc"], ctx["tc"], ctx["ident"]
    n_kv = 8 if memT is None else 16
    with tc.tile_pool(name="attn_sb", bufs=1) as sb:
        w = sb.tile([128, 3, 2, 8, 512], bf16, tag="w")
        nc.sync.dma_start(w[:, 1:3], wqkv[:, 1:3])
        nc.sync.dma_start(w[:, 0:1], wqkv[:, 0:1])
        wo = sb.tile([128, 8, D], bf16, tag="wo")
        nc.sync.dma_start(wo[:], wot[:])

        with tc.tile_pool(name="attn_ps", bufs=1, space="PSUM") as ps:
            for b in range(BPC):
                # ---- lhsT sources ----
                if xq_dram is not None:
                    xq = sb.tile([128, 8, S_T], bf16, tag="xq", bufs=2)
                    nc.scalar.dma_start(xq[:], xq_dram[b])
                else:
                    xq = xT_in[b]

                xt_next = xt_alloc()
                xT_out[b] = xt_next
                for hg in range(2):
                    # ---- stage A: K/V proj + exp/evac + A accumulation ----
                    expk = sb.tile([128, n_kv, 512], bf16, tag="expk")
                    expv = sb.tile([128, n_kv, 512], bf16, tag="expv")
                    for j in range(n_kv // 2):
                        if memT is not None:
                            mt = sb.tile([128, 2, 8, 128], bf16, tag="mt",
                                         bufs=2)
                            nc.scalar.dma_start(mt[:], memT[b, j])
                        for i in range(2):
                            sm = 2 * j + i
                            kps = ps.tile([128, 512], f32, tag="ps512", bufs=3)
                            vps = ps.tile([128, 512], f32, tag="ps512", bufs=3)
                            for k in range(8):
                                if memT is None:
                                    lhsT = xq[:, k, 128 * sm:128 * (sm + 1)]
                                else:
                                    lhsT = mt[:, i, k, :]
                                nc.tensor.matmul(kps[:], lhsT, w[:, 1, hg, k, :],
                                                 start=(k == 0), stop=(k == 7))
                                nc.tensor.matmul(vps[:], lhsT, w[:, 2, hg, k, :],
                                                 start=(k == 0), stop=(k == 7))
                            nc.scalar.activation(expk[:, sm, :], kps[:], AF.Exp,
                                                 scale=1.0 / SCALE)
                            krs = sb.tile([128, 4], f32, tag="krs", bufs=2)
                            nc.vector.tensor_reduce(
                                out=krs[:],
                                in_=expk[:, sm, :].rearrange("p (h q) -> p h q", h=4),
                                axis=mybir.AxisListType.X, op=ALU.add)
                            krr = sb.tile([128, 4], f32, tag="krr", bufs=2)
                            nc.vector.reciprocal(krr[:], krs[:])
                            nc.vector.tensor_tensor(
                                out=expv[:, sm, :].rearrange("p (h q) -> p h q", h=4),
                                in0=vps[:].rearrange("p (h q) -> p h q", h=4),
                                in1=krr[:].unsqueeze(2).broadcast_to([128, 4, 128]),
                                op=ALU.mult)
                    # per-region accumulation groups must stay consecutive:
                    # interleaving groups within one PSUM bank corrupts them.
                    aps = ps.tile([128, 512], f32, tag="aps", bufs=1)
                    for hi in range(4):
                        for sm in range(n_kv):
                            nc.tensor.matmul(
                                aps[:, 128 * hi:128 * (hi + 1)],
                                expk[:, sm, 128 * hi:128 * (hi + 1)],
                                expv[:, sm, 128 * hi:128 * (hi + 1)],
                                start=(sm == 0), stop=(sm == n_kv - 1))
                    asb = sb.tile([128, 512], bf16, tag="asb", bufs=2)
                    nc.vector.tensor_copy(asb[:], aps[:])

                    # ---- stage B: Q proj for all st, then all transposes ----
                    softqT = sb.tile([128, 4, S_T], bf16, tag="softqT", bufs=1)
                    sqa = sb.tile([128, 8, 4, 128], bf16, tag="sqa", bufs=1)
                    for st in range(8):
                        qps = ps.tile([128, 512], f32, tag="ps512", bufs=3)
                        for k in range(8):
                            nc.tensor.matmul(
                                qps[:], xq[:, k, 128 * st:128 * (st + 1)],
                                w[:, 0, hg, k, :], start=(k == 0), stop=(k == 7))
                        if masked and st == 0:
                            nc.vector.tensor_tensor(
                                out=qps[:], in0=qps[:], in1=ctx["maskt"][:],
                                op=ALU.add)
                        eq = sb.tile([128, 512], f32, tag="eq", bufs=2)
                        nc.scalar.activation(eq[:], qps[:], AF.Exp,
                                             scale=1.0 / SCALE)
                        qrs = sb.tile([128, 4], f32, tag="qrs", bufs=2)
                        nc.vector.tensor_reduce(
                            out=qrs[:], in_=eq[:].rearrange("p (h q) -> p h q", h=4),
                            axis=mybir.AxisListType.X, op=ALU.add)
                        qrr = sb.tile([128, 4], f32, tag="qrr", bufs=2)
                        nc.vector.reciprocal(qrr[:], qrs[:])
                        nc.vector.tensor_tensor(
                            out=sqa[:, st], in0=eq[:].rearrange("p (h q) -> p h q", h=4),
                            in1=qrr[:].unsqueeze(2).broadcast_to([128, 4, 128]),
                            op=ALU.mult)
                    for st in range(8):
                        tp = ps.tile([128, 8, 128], bf16, tag="tpb", bufs=2)
                        for hi in range(4):
                            nc.tensor.transpose(tp[:, hi, :], sqa[:, st, hi, :],
                                                ident[:])
                        nc.scalar.activation(
                            softqT[:, :, 128 * st:128 * (st + 1)],
                            tp[:, 0:4, :], AF.Identity)

                    # ---- stage C: Bm, Wo, residual + LN per head ----
                    nats, ybs = [], []
                    for hi in range(4):
                        hb = 4 * hg + hi
                        nat = sb.tile([128, D], bf16, tag="res_nat", bufs=3)
                        nc.scalar.dma_start(
                            nat[:], res_d[b, 128 * hb:128 * (hb + 1), :])
                        nats.append(nat)
                    for hi in range(4):
                        hb = 4 * hg + hi  # head == output s-tile block
                        bms = sb.tile([128, S_T], bf16, tag="bms", bufs=2)
                        for half in range(2):
                            bmt = ps.tile([128, 512], f32, tag="ps512", bufs=3)
                            nc.tensor.matmul(bmt[:],
                                             asb[:, 128 * hi:128 * (hi + 1)],
                                             softqT[:, hi,
                                                    512 * half:512 * (half + 1)])
                            nc.vector.tensor_copy(
                                bms[:, 512 * half:512 * (half + 1)], bmt[:])
                        ops = ps.tile([128, D], f32, tag="ps1k", bufs=1)
                        for jj in range(8):
                            for nh in range(2):
                                nc.tensor.matmul(
                                    ops[:, 512 * nh:512 * (nh + 1)],
                                    bms[:, jj::8],
                                    wo[:, jj, 512 * nh:512 * (nh + 1)],
                                    start=(jj == 0), stop=(jj == 7))
                        rsd = sb.tile([128, D], f32, tag="rsd", bufs=1)
                        nc.vector.tensor_tensor(out=rsd[:], in0=ops[:],
                                                in1=nats[hi][:], op=ALU.add)
                        yb = _layernorm(ctx, sb, rsd,
                                        y_next_d[b, 128 * hb:128 * (hb + 1), :],
                                        gbi, bf16)
                        ybs.append(yb)
                    # transposed copies for the next phase, batched so the
                    # PE never waits on an LN chain mid-stage
                    for hi in range(4):
                        hb = 4 * hg + hi
                        tp2 = ps.tile([128, 8, 128], bf16, tag="tpb", bufs=2)
                        for k in range(8):
                            nc.tensor.transpose(
                                tp2[:, k, :],
                                ybs[hi][:, 128 * k:128 * (k + 1)], ident[:])
                        nc.scalar.activation(
                            xt_next[:, :, 128 * hb:128 * (hb + 1)],
                            tp2[:], AF.Identity)


def _phase_lffn(ctx, y2T, e1w_d, d1w_d, e2w_d, d2w_d, y2d, out, gbi):
    nc, tc = ctx["nc"], ctx["tc"]
    with tc.tile_pool(name="ffn_sb", bufs=1) as sb:
        e1 = sb.tile([128, 8, 4, 128], bf16, tag="e1")
        nc.sync.dma_start(e1[:], e1w_d[:])
        d1 = sb.tile([128, 4, 8, 128], bf16, tag="d1")
        nc.sync.dma_start(d1[:], d1w_d[:])
        e2 = sb.tile([128, 8, 4, 128], bf16, tag="e2")
        nc.sync.dma_start(e2[:], e2w_d[:])
        d2 = sb.tile([128, 4, D], bf16, tag="d2")
        nc.sync.dma_start(d2[:], d2w_d[:])

        with tc.tile_pool(name="ffn_ps", bufs=1, space="PSUM") as ps:
            for b in range(BPC):
                xT = y2T[b]  # [128, 8, S_T]
                # h1T = E1 @ y2T  [BN(4 tiles), S_T]
                h1T = [sb.tile([128, S_T], bf16, tag=f"h1T{t_}",
                               name=f"h1T{t_}") for t_ in range(4)]
                for t_ in range(4):
                    acc = ps.tile([128, S_T], f32, tag="acc", bufs=3)
                    for nh in range(2):
                        for k in range(8):
                            nc.tensor.matmul(
                                acc[:, 512 * nh:512 * (nh + 1)], e1[:, k, t_, :],
                                xT[:, k, 512 * nh:512 * (nh + 1)],
                                start=(k == 0), stop=(k == 7))
                    nc.vector.tensor_copy(h1T[t_][:], acc[:])
                # h2T = D1 @ h1T -> silu -> swT  [HID(8 tiles), S_T]
                swT = [sb.tile([128, S_T], bf16, tag=f"swT{t_}",
                               name=f"swT{t_}") for t_ in range(8)]
                for t_ in range(8):
                    acc = ps.tile([128, S_T], f32, tag="acc", bufs=3)
                    for nh in range(2):
                        for k in range(4):
                            nc.tensor.matmul(
                                acc[:, 512 * nh:512 * (nh + 1)], d1[:, k, t_, :],
                                h1T[k][:, 512 * nh:512 * (nh + 1)],
                                start=(k == 0), stop=(k == 3))
                    nc.scalar.activation(swT[t_][:], acc[:], AF.Silu)
                # g1T = E2 @ swT  [BN(4 tiles), S_T]
                g1T = [sb.tile([128, S_T], bf16, tag=f"g1T{t_}",
                               name=f"g1T{t_}") for t_ in range(4)]
                for t_ in range(4):
                    acc = ps.tile([128, S_T], f32, tag="acc", bufs=3)
                    for nh in range(2):
                        for k in range(8):
                            nc.tensor.matmul(
                                acc[:, 512 * nh:512 * (nh + 1)], e2[:, k, t_, :],
                                swT[k][:, 512 * nh:512 * (nh + 1)],
                                start=(k == 0), stop=(k == 7))
                    nc.vector.tensor_copy(g1T[t_][:], acc[:])
                # ffn[st] = g1T[:, st].T @ D2T ; residual with y2, LN3 -> out
                for st in range(8):
                    nat = sb.tile([128, D], bf16, tag="y2res", bufs=2)
                    nc.sync.dma_start(nat[:],
                                      y2d[b, 128 * st:128 * (st + 1), :])
                    acc = ps.tile([128, D], f32, tag="acc2", bufs=1)
                    for nh in range(2):
                        for k in range(4):
                            nc.tensor.matmul(
                                acc[:, 512 * nh:512 * (nh + 1)],
                                g1T[k][:, 128 * st:128 * (st + 1)],
                                d2[:, k, 512 * nh:512 * (nh + 1)],
                                start=(k == 0), stop=(k == 3))
                    rsd = sb.tile([128, D], f32, tag="rsd", bufs=2)
                    nc.vector.tensor_tensor(out=rsd[:], in0=acc[:], in1=nat[:],
                                            op=ALU.add)
                    _layernorm(ctx, sb, rsd,
                               out[b, 128 * st:128 * (st + 1), :], gbi, f32)


_CACHE = {}


def _prep_host(inputs):
    """Convert/transpose/pack weights + activations per the kernel layout."""
    g = {k: np.asarray(v) for k, v in inputs.items()}
    affine = not (
        np.all(g["g1"] == 1) and np.all(g["g2"] == 1) and np.all(g["g3"] == 1)
        and np.all(g["b1"] == 0) and np.all(g["b2"] == 0) and np.all(g["b3"] == 0))

    def wqkv_pack(q, k, v):
        # [H, D, DQ] -> [p=128][qkv][hg][kchunk][512] (4 heads concat)
        def onev2(w):
            arr = np.empty((2, 8, 128, 512), np.float32)
            for hg in range(2):
                for kc in range(8):
                    cols = [w[4 * hg + hi, 128 * kc:128 * (kc + 1), :]
                            for hi in range(4)]
                    arr[hg, kc] = np.concatenate(cols, axis=1)
            return arr
        st = np.stack([onev2(q), onev2(k), onev2(v)])  # [3,2,8,128,512]
        return np.ascontiguousarray(st.transpose(3, 0, 1, 2, 4)).astype(bf)

    host = {}
    host["wqkv1"] = wqkv_pack(g["Wq1"], g["Wk1"], g["Wv1"])
    host["wqkv2"] = wqkv_pack(g["Wq2"], g["Wk2"], g["Wv2"])
    host["wo1t"] = np.ascontiguousarray(
        g["Wo1"].T.reshape(8, 128, D).transpose(1, 0, 2)).astype(bf)
    host["wo2t"] = np.ascontiguousarray(
        g["Wo2"].T.reshape(8, 128, D).transpose(1, 0, 2)).astype(bf)
    host["e1w"] = np.ascontiguousarray(
        g["E1"].T.reshape(8, 128, 4, 128).transpose(1, 0, 2, 3)).astype(bf)
    host["d1w"] = np.ascontiguousarray(
        g["D1"].T.reshape(4, 128, 8, 128).transpose(1, 0, 2, 3)).astype(bf)
    host["e2w"] = np.ascontiguousarray(
        g["E2"].T.reshape(8, 128, 4, 128).transpose(1, 0, 2, 3)).astype(bf)
    host["d2w"] = np.ascontiguousarray(
        g["D2"].T.reshape(4, 128, D).transpose(1, 0, 2)).astype(bf)
    mask = np.where(np.arange(DQ)[None, :] <= np.arange(128)[:, None],
                    0.0, NEG).astype(np.float32)
    host["mask4"] = np.tile(mask, (1, 4))
    if affine:
        host["grep"] = np.stack([
            np.broadcast_to(g[n].astype(np.float32), (128, D))
            for n in ("g1", "b1", "g2", "b2", "g3", "b3")]).copy()

    in_maps = []
    y = g["y"].astype(np.float32)
    mem = g["mem"].astype(np.float32)
    for c in range(N_CORES):
        sl = slice(BPC * c, BPC * (c + 1))
        m = dict(host)
        m["y0b"] = y[sl].astype(bf)
        yT = np.ascontiguousarray(y[sl].transpose(0, 2, 1)).astype(bf)
        m["y0T"] = np.ascontiguousarray(
            yT.reshape(BPC, 8, 128, S_T).transpose(0, 2, 1, 3))
        mT = mem[sl].transpose(0, 2, 1).astype(bf)  # [b, D, S_M]
        # [b, k, p, j, i, q] -> [b, j, p, i, k, q]
        m["memTp"] = np.ascontiguousarray(
            mT.reshape(BPC, 8, 128, 8, 2, 128).transpose(0, 3, 2, 4, 1, 5))
        in_maps.append(m)
    return in_maps, affine


def kernel(**inputs):
    in_maps, affine = _prep_host(inputs)
    if affine not in _CACHE:
        _CACHE[affine] = _build(affine)
    nc = _CACHE[affine]
    res = run_bass_kernel_spmd(nc, in_maps, list(range(N_CORES)))
    return np.concatenate([r["out"] for r in res.results], axis=0)


if __name__ == "__main__":
    rng = np.random.default_rng(0)
    ins = {
        "mem": rng.standard_normal((B, S_M, D), dtype=np.float32),
        "y": rng.standard_normal((B, S_T, D), dtype=np.float32),
        **{k: (rng.standard_normal(s, dtype=np.float32) * 0.02).astype(np.float32)
           for k, s in {
               "Wq1": (H, D, DQ), "Wk1": (H, D, DQ), "Wv1": (H, D, DQ),
               "Wo1": (D, D), "Wq2": (H, D, DQ), "Wk2": (H, D, DQ),
               "Wv2": (H, D, DQ), "Wo2": (D, D), "E1": (BNK, D),
               "D1": (HID, BNK), "E2": (BNK, HID), "D2": (D, BNK)}.items()},
        "g1": np.ones(D, np.float32), "b1": np.zeros(D, np.float32),
        "g2": np.ones(D, np.float32), "b2": np.zeros(D, np.float32),
        "g3": np.ones(D, np.float32), "b3": np.zeros(D, np.float32),
    }
    o = kernel(**ins)
    print("out", o.shape, o.dtype, np.abs(o).mean())
